# revision 4
# baseline (speedup 1.0000x reference)
"""CompressedFP8Linear on 8 trn2 NeuronCores.

out[B,S,O] = x @ (weight * weight_scale).T + bias
  x:[4,32,8192] f32, weight:[8192,8192] f32 (fp8-e4m3 representable),
  weight_scale:[8192,1] f32, bias:[8192] f16.

Strategy (column-parallel, per sharding hint):
  - Shard weight rows (out_features) across 8 cores; replicate x.
  - The reference round-trips weight through fp8-e4m3, so every weight
    value is EXACTLY representable in TRN float8e4 (|w| <= ~5.5 << 240).
    Host converts the weight shard to fp8 and ships 8 MiB/core instead
    of 32 MiB -- a 4x cut of the dominant DMA stream.
  - x is shipped as fp16 (quantization rel err ~2e-4, budget 2e-2).
    The PE runs MIXED-dtype matmuls: fp16 stationary (x) x fp8 moving
    (weight), both upconverted to FP22 internally; fp8 streams at bf16
    rate so the ALU floor is 64 kt x 2 x 512 cycles ~ 27.5 us/core.
  - Host-side marshalling (layout only): weight shard transposed to
    [p, kt, o] and x packed to [p, kt, m] so every SBUF partition's DMA
    reads are contiguous DRAM runs.
  - scale/bias arrive as [1, O_shard] f32 rows, broadcast on-chip to the
    128 token partitions via an exact ones-outer-product on the (idle)
    PE.  Dequant scale is applied to the [128, O] output (64x fewer
    multiplies than dequantizing the weight), bias added, and the result
    is written back as fp16 (another ~1e-4 error, halves out traffic).
  - No collectives; the host concatenates the 8 output shards.

Memory floor per core: 8 MiB weight + 2 MiB x + 0.25 MiB out ~ 10.6 MiB.
"""

import numpy as np
import ml_dtypes

import concourse.bass as bass
import concourse.mybir as mybir
import concourse.tile as tile
from concourse.bass_utils import run_bass_kernel_spmd

B, S, IN, OUT = 4, 32, 8192, 8192
M = B * S                      # 128 tokens
NCORES = 8
OSH = OUT // NCORES            # 1024 out-features per core
KT = IN // 128                 # 64 k-tiles
F32 = mybir.dt.float32
F16 = mybir.dt.float16
F8 = mybir.dt.float8e4         # numpy side: ml_dtypes.float8_e4m3


def split_waits(nc, max_waits=1):
    """This walrus build encodes at most one sem-wait per instruction;
    move any excess onto NoOps injected just before (same engine queue,
    so ordering semantics are identical)."""
    n = 0
    for f in nc.m.functions:
        for bb in f.blocks:
            out = []
            for inst in bb.instructions:
                si = inst.sync_info
                waits = list(si.on_wait) if si and si.on_wait else []
                if len(waits) > max_waits:
                    extra, keep = waits[:-max_waits], waits[-max_waits:]
                    for i, w in enumerate(extra):
                        out.append(mybir.InstNoOp(
                            name=f"{inst.name}-ws{i}", engine=inst.engine,
                            ins=[], outs=[],
                            sync_info=mybir.SyncInfo(on_wait=[w], on_update=[])))
                        n += 1
                    si.on_wait = keep
                out.append(inst)
            bb.instructions = out
    return n


def _make_slabs(slab_kt):
    """Big slabs for DMA stream efficiency, tapered at the end so the final
    data->matmul->store dependency chain is short."""
    slabs = []
    k0 = 0
    while k0 < KT - slab_kt:
        slabs.append((k0, slab_kt))
        k0 += slab_kt
    while k0 < KT:
        n = max(1, min(4, KT - k0 - 2))
        slabs.append((k0, n))
        k0 += n
    return slabs


def _declare(nc):
    # host-packed layouts: each SBUF partition's DMA is one contiguous run
    xt_d = nc.dram_tensor("xt", [128, KT, M], F16, kind="ExternalInput")
    wt_d = nc.dram_tensor("wt", [128, KT, OSH], F8, kind="ExternalInput")
    sc_d = nc.dram_tensor("scale_r", [1, OSH], F32, kind="ExternalInput")
    bi_d = nc.dram_tensor("bias_r", [1, OSH], F32, kind="ExternalInput")
    out_d = nc.dram_tensor("out", [M, OSH], F16, kind="ExternalOutput")
    return xt_d, wt_d, sc_d, bi_d, out_d


def _emit_prologue(nc, cp, ps, x_eng, sc_d, bi_d):
    """Broadcast scale/bias rows to all 128 partitions on-chip: exact fp32
    outer product with a ones column on the (still idle) PE, instead of
    streaming 1 MiB of replicated data."""
    ones = cp.tile([1, M], F32)
    nc.vector.memset(ones[:], 1.0)
    sc = cp.tile([M, OSH], F32)
    bi = cp.tile([M, OSH], F32)
    for row_d, dst in ((sc_d, sc), (bi_d, bi)):
        row = cp.tile([1, OSH], F32, tag="crow")
        x_eng.dma_start(row[:], row_d[:])
        pb = ps.tile([M, OSH], F32, tag="pbcast")
        for og in range(2):
            nc.tensor.matmul(
                pb[:, og * 512:(og + 1) * 512],
                ones[:, :], row[:, og * 512:(og + 1) * 512],
                start=True, stop=True)
        nc.vector.tensor_copy(dst[:], pb[:])
    return sc, bi


def _emit_rep(nc, pools, engines, tensors, sc, bi, slabs, slab_kt, x_chunks):
    """One full shard computation: out[128, OSH] = (xT.T @ WT)*scale + bias."""
    xp, wp, op, ps = pools
    x_eng, w_engs = engines
    xt_d, wt_d, out_d = tensors

    # x: 2 MiB in chunks so the first matmul waits only ~0.25 MiB
    xsb = xp.tile([128, KT, M], F16)
    per = KT // x_chunks
    for i in range(x_chunks):
        x_eng.dma_start(
            xsb[:, i * per:(i + 1) * per, :],
            xt_d[:, i * per:(i + 1) * per, :])

    acc0 = ps.tile([M, 512], F32)
    acc1 = ps.tile([M, 512], F32)
    accs = (acc0, acc1)
    for t, (k0, n) in enumerate(slabs):
        wsb = wp.tile([128, slab_kt, OSH], F8, tag="wsb")
        # spread weight DMAs over queues so they pipeline
        w_engs[t % len(w_engs)].dma_start(
            wsb[:, :n, :], wt_d[:, k0:k0 + n, :])
        for s in range(n):
            k = k0 + s
            for og in range(2):
                nc.tensor.matmul(
                    accs[og][:, :],
                    xsb[:, k, :],
                    wsb[:, s, og * 512:(og + 1) * 512],
                    start=(k == 0), stop=(k == KT - 1))

    tmp = op.tile([M, OSH], F32, tag="tmp32")
    outsb = op.tile([M, OSH], F16, tag="o16")
    for og in range(2):
        osl = slice(og * 512, (og + 1) * 512)
        nc.vector.tensor_mul(tmp[:, osl], accs[og][:, :], sc[:, osl])
        nc.vector.tensor_add(outsb[:, osl], tmp[:, osl], bi[:, osl])
        # write each half back as soon as its scale/bias is done
        x_eng.dma_start(out_d[:, osl], outsb[:, osl])


def build(reps=1, slab_kt=8, w_engines=("scalar",), x_engine="sync",
          x_chunks=8, loops=0, wp_bufs=4):
    """One column-parallel shard.

    reps > 1 unrolls the whole body (including all DMA) back-to-back for
    steady-state timing; the computation is identical each rep.
    loops > 0 wraps the reps in a hardware For_i loop executing the body
    `loops` times (for low-noise on-device timing; same data each pass).
    """
    nc = bass.Bass()
    xt_d, wt_d, sc_d, bi_d, out_d = _declare(nc)
    slabs = _make_slabs(slab_kt)

    with tile.TileContext(nc) as tc:
        with (
            tc.tile_pool(name="xp", bufs=2) as xp,
            tc.tile_pool(name="wp", bufs=wp_bufs) as wp,
            tc.tile_pool(name="cp", bufs=1) as cp,
            tc.tile_pool(name="op", bufs=2) as op,
            tc.tile_pool(name="ps", bufs=2, space="PSUM") as ps,
        ):
            x_eng = getattr(nc, x_engine)
            w_engs = [getattr(nc, e) for e in w_engines]
            sc, bi = _emit_prologue(nc, cp, ps, x_eng, sc_d, bi_d)

            pools = (xp, wp, op, ps)
            engines = (x_eng, w_engs)
            tensors = (xt_d, wt_d, out_d)

            def emit_reps():
                for _ in range(reps):
                    _emit_rep(nc, pools, engines, tensors, sc, bi,
                              slabs, slab_kt, x_chunks)

            if loops > 0:
                with tc.For_i(0, loops):
                    emit_reps()
            else:
                emit_reps()

    split_waits(nc)
    return nc


def shard_inputs(x, weight, weight_scale, bias):
    """Host-side marshalling into per-core input maps (layout + dtype only;
    the fp8 weight conversion is exact because the reference round-trips
    weight through fp8-e4m3)."""
    x = np.asarray(x, dtype=np.float32)
    weight = np.asarray(weight, dtype=np.float32)
    scale = np.asarray(weight_scale, dtype=np.float32).reshape(OUT)
    bias32 = np.asarray(bias).astype(np.float32)

    # pack x as [p, kt, m] (k = kt*128 + p) so each SBUF partition's x data
    # is one contiguous DRAM run
    xt = np.ascontiguousarray(
        np.transpose(x.reshape(M, KT, 128), (2, 1, 0))).astype(np.float16)
    in_maps = []
    for c in range(NCORES):
        sl = slice(c * OSH, (c + 1) * OSH)
        # wt[p, kt, o] = W_shard[o, kt*128 + p]  (k-major on partitions)
        wt = np.ascontiguousarray(
            weight[sl, :].T.reshape(KT, 128, OSH).transpose(1, 0, 2)
        ).astype(ml_dtypes.float8_e4m3)
        in_maps.append({
            "xt": xt, "wt": wt,
            "scale_r": np.ascontiguousarray(scale[sl][None, :]),
            "bias_r": np.ascontiguousarray(bias32[sl][None, :]),
        })
    return in_maps


def kernel(x, weight, weight_scale, bias):
    nc = build(reps=1)
    in_maps = shard_inputs(x, weight, weight_scale, bias)
    res = run_bass_kernel_spmd(nc, in_maps, core_ids=list(range(NCORES)))
    out = np.concatenate(
        [np.asarray(res.results[c]["out"]) for c in range(NCORES)], axis=1)
    return out.astype(np.float32).reshape(B, S, OUT)


# revision 9
# speedup vs baseline: 1.2752x; 1.2752x over previous
"""CompressedFP8Linear on 8 trn2 NeuronCores.

out[B,S,O] = x @ (weight * weight_scale).T + bias
  x:[4,32,8192] f32, weight:[8192,8192] f32 (fp8-e4m3 representable),
  weight_scale:[8192,1] f32, bias:[8192] f16.

Strategy (column-parallel, per sharding hint):
  - Shard weight rows (out_features) across 8 cores; replicate x.
  - The reference round-trips weight through fp8-e4m3, so every weight
    value is EXACTLY representable in TRN float8e4 (|w| <= ~5.5 << 240).
    Host converts the weight shard to fp8 and ships 8 MiB/core instead
    of 32 MiB -- a 4x cut of the dominant DMA stream.
  - x is shipped as fp16 (quantization rel err ~2e-4, budget 2e-2).
    The PE runs MIXED-dtype matmuls: fp16 stationary (x) x fp8 moving
    (weight), both upconverted to FP22 internally; fp8 streams at bf16
    rate so the ALU floor is 64 kt x 2 x 512 cycles ~ 27.5 us/core.
  - Host-side marshalling (layout only): weight shard transposed to
    [p, kt, o] and x packed to [p, kt, m] so every SBUF partition's DMA
    reads are contiguous DRAM runs.
  - scale/bias arrive as [1, O_shard] f32 rows, broadcast on-chip to the
    128 token partitions via an exact ones-outer-product on the (idle)
    PE.  Dequant scale is applied to the [128, O] output (64x fewer
    multiplies than dequantizing the weight), bias added, and the result
    is written back as fp16 (another ~1e-4 error, halves out traffic).
  - No collectives; the host concatenates the 8 output shards.

Memory floor per core: 8 MiB weight + 2 MiB x + 0.25 MiB out ~ 10.6 MiB.
"""

import numpy as np
import ml_dtypes

import concourse.bass as bass
import concourse.mybir as mybir
import concourse.tile as tile
from concourse.bass_utils import run_bass_kernel_spmd

B, S, IN, OUT = 4, 32, 8192, 8192
M = B * S                      # 128 tokens
NCORES = 8
OSH = OUT // NCORES            # 1024 out-features per core
KT = IN // 128                 # 64 k-tiles
F32 = mybir.dt.float32
F16 = mybir.dt.float16
F8 = mybir.dt.float8e4         # numpy side: ml_dtypes.float8_e4m3


def split_waits(nc, max_waits=1):
    """This walrus build encodes at most one sem-wait per instruction;
    move any excess onto NoOps injected just before (same engine queue,
    so ordering semantics are identical)."""
    n = 0
    for f in nc.m.functions:
        for bb in f.blocks:
            out = []
            for inst in bb.instructions:
                si = inst.sync_info
                waits = list(si.on_wait) if si and si.on_wait else []
                if len(waits) > max_waits:
                    extra, keep = waits[:-max_waits], waits[-max_waits:]
                    for i, w in enumerate(extra):
                        out.append(mybir.InstNoOp(
                            name=f"{inst.name}-ws{i}", engine=inst.engine,
                            ins=[], outs=[],
                            sync_info=mybir.SyncInfo(on_wait=[w], on_update=[])))
                        n += 1
                    si.on_wait = keep
                out.append(inst)
            bb.instructions = out
    return n


def _make_slabs(slab_kt):
    """Big slabs for DMA stream efficiency, tapered at the end so the final
    data->matmul->store dependency chain is short."""
    slabs = []
    k0 = 0
    while k0 < KT - slab_kt:
        slabs.append((k0, slab_kt))
        k0 += slab_kt
    while k0 < KT:
        n = max(1, min(4, KT - k0 - 2))
        slabs.append((k0, n))
        k0 += n
    return slabs


def _declare(nc):
    # host-packed layouts: each SBUF partition's DMA is one contiguous run
    xt_d = nc.dram_tensor("xt", [128, KT, M], F16, kind="ExternalInput")
    wt_d = nc.dram_tensor("wt", [128, KT, OSH], F8, kind="ExternalInput")
    sc_d = nc.dram_tensor("scale_r", [1, OSH], F32, kind="ExternalInput")
    bi_d = nc.dram_tensor("bias_r", [1, OSH], F32, kind="ExternalInput")
    out_d = nc.dram_tensor("out", [M, OSH], F16, kind="ExternalOutput")
    return xt_d, wt_d, sc_d, bi_d, out_d


def _emit_prologue(nc, cp, psb, x_eng, sc_d, bi_d):
    """Broadcast scale/bias rows to all 128 partitions on-chip: exact fp32
    outer product with a ones column on the (still idle) PE, instead of
    streaming 1 MiB of replicated data."""
    ones = cp.tile([1, M], F32)
    nc.vector.memset(ones[:], 1.0)
    sc = cp.tile([M, OSH], F32)
    bi = cp.tile([M, OSH], F32)
    for row_d, dst in ((sc_d, sc), (bi_d, bi)):
        row = cp.tile([1, OSH], F32, tag="crow")
        x_eng.dma_start(row[:], row_d[:])
        pb = psb.tile([M, OSH], F32, tag="pbcast")
        for og in range(2):
            nc.tensor.matmul(
                pb[:, og * 512:(og + 1) * 512],
                ones[:, :], row[:, og * 512:(og + 1) * 512],
                start=True, stop=True)
        nc.vector.tensor_copy(dst[:], pb[:])
    return sc, bi


def _emit_rep(nc, pools, engines, tensors, sc, bi, slabs, slab_kt, x_chunks):
    """One full shard computation: out[128, OSH] = (xT.T @ WT)*scale + bias.

    Queue discipline: ALL pure-prefetch traffic (x chunks + W slabs,
    interleaved in consumption order) goes on one HWDGE queue; the
    dependent out-DMAs go on the other.  HWDGE queues are FIFO per
    engine, so a compute-dependent DMA in the prefetch queue would stall
    the next rep's entire fill behind this rep's compute tail.
    """
    xp, wp, op, ps = pools
    stream_eng, out_eng = engines
    xt_d, wt_d, out_d = tensors

    xsb = xp.tile([128, KT, M], F16)
    x_per = KT // x_chunks

    def emit_x_chunk(i):
        stream_eng.dma_start(
            xsb[:, i * x_per:(i + 1) * x_per, :],
            xt_d[:, i * x_per:(i + 1) * x_per, :])

    # separate tags so consecutive reps alternate over 4 PSUM banks --
    # measured 177 ns/MM vs 283 ns/MM with a shared 2-bank rotation
    acc0 = ps.tile([M, 512], F32, tag="a0")
    acc1 = ps.tile([M, 512], F32, tag="a1")
    accs = (acc0, acc1)
    next_x = 0
    for t, (k0, n) in enumerate(slabs):
        # keep the x stream just ahead of the k-tiles that consume it
        while next_x < x_chunks and next_x * x_per <= k0 + n:
            emit_x_chunk(next_x)
            next_x += 1
        wsb = wp.tile([128, slab_kt, OSH], F8, tag="wsb")
        stream_eng.dma_start(wsb[:, :n, :], wt_d[:, k0:k0 + n, :])
        for s in range(n):
            k = k0 + s
            for og in range(2):
                nc.tensor.matmul(
                    accs[og][:, :],
                    xsb[:, k, :],
                    wsb[:, s, og * 512:(og + 1) * 512],
                    start=(k == 0), stop=(k == KT - 1))
    while next_x < x_chunks:
        emit_x_chunk(next_x)
        next_x += 1

    tmp = op.tile([M, OSH], F32, tag="tmp32")
    outsb = op.tile([M, OSH], F16, tag="o16")
    for og in range(2):
        osl = slice(og * 512, (og + 1) * 512)
        nc.vector.tensor_mul(tmp[:, osl], accs[og][:, :], sc[:, osl])
        nc.vector.tensor_add(outsb[:, osl], tmp[:, osl], bi[:, osl])
        # write each half back as soon as its scale/bias is done
        out_eng.dma_start(out_d[:, osl], outsb[:, osl])


def build(reps=1, slab_kt=4, stream_engine="scalar", out_engine="sync",
          x_chunks=8, loops=0, wp_bufs=8):
    """One column-parallel shard.

    reps > 1 unrolls the whole body (including all DMA) back-to-back for
    steady-state timing; the computation is identical each rep.
    loops > 0 wraps the reps in a hardware For_i loop executing the body
    `loops` times (for low-noise on-device timing; same data each pass).
    """
    nc = bass.Bass()
    xt_d, wt_d, sc_d, bi_d, out_d = _declare(nc)
    slabs = _make_slabs(slab_kt)

    with tile.TileContext(nc) as tc:
        with (
            tc.tile_pool(name="xp", bufs=2) as xp,
            tc.tile_pool(name="wp", bufs=wp_bufs) as wp,
            tc.tile_pool(name="cp", bufs=1) as cp,
            tc.tile_pool(name="op", bufs=2) as op,
            tc.tile_pool(name="psb", bufs=1, space="PSUM") as psb,
            tc.tile_pool(name="ps", bufs=2, space="PSUM") as ps,
        ):
            stream_eng = getattr(nc, stream_engine)
            out_eng = getattr(nc, out_engine)
            sc, bi = _emit_prologue(nc, cp, psb, out_eng, sc_d, bi_d)

            pools = (xp, wp, op, ps)
            engines = (stream_eng, out_eng)
            tensors = (xt_d, wt_d, out_d)

            def emit_reps():
                for _ in range(reps):
                    _emit_rep(nc, pools, engines, tensors, sc, bi,
                              slabs, slab_kt, x_chunks)

            if loops > 0:
                with tc.For_i(0, loops):
                    emit_reps()
            else:
                emit_reps()

    split_waits(nc)
    return nc


def shard_inputs(x, weight, weight_scale, bias):
    """Host-side marshalling into per-core input maps (layout + dtype only;
    the fp8 weight conversion is exact because the reference round-trips
    weight through fp8-e4m3)."""
    x = np.asarray(x, dtype=np.float32)
    weight = np.asarray(weight, dtype=np.float32)
    scale = np.asarray(weight_scale, dtype=np.float32).reshape(OUT)
    bias32 = np.asarray(bias).astype(np.float32)

    # pack x as [p, kt, m] (k = kt*128 + p) so each SBUF partition's x data
    # is one contiguous DRAM run
    xt = np.ascontiguousarray(
        np.transpose(x.reshape(M, KT, 128), (2, 1, 0))).astype(np.float16)
    in_maps = []
    for c in range(NCORES):
        sl = slice(c * OSH, (c + 1) * OSH)
        # wt[p, kt, o] = W_shard[o, kt*128 + p]  (k-major on partitions)
        wt = np.ascontiguousarray(
            weight[sl, :].T.reshape(KT, 128, OSH).transpose(1, 0, 2)
        ).astype(ml_dtypes.float8_e4m3)
        in_maps.append({
            "xt": xt, "wt": wt,
            "scale_r": np.ascontiguousarray(scale[sl][None, :]),
            "bias_r": np.ascontiguousarray(bias32[sl][None, :]),
        })
    return in_maps


def kernel(x, weight, weight_scale, bias):
    nc = build(reps=1)
    in_maps = shard_inputs(x, weight, weight_scale, bias)
    res = run_bass_kernel_spmd(nc, in_maps, core_ids=list(range(NCORES)))
    out = np.concatenate(
        [np.asarray(res.results[c]["out"]) for c in range(NCORES)], axis=1)
    return out.astype(np.float32).reshape(B, S, OUT)


# revision 10
# speedup vs baseline: 1.2785x; 1.0026x over previous
"""CompressedFP8Linear on 8 trn2 NeuronCores.

out[B,S,O] = x @ (weight * weight_scale).T + bias
  x:[4,32,8192] f32, weight:[8192,8192] f32 (fp8-e4m3 representable),
  weight_scale:[8192,1] f32, bias:[8192] f16.

Strategy (column-parallel, per sharding hint):
  - Shard weight rows (out_features) across 8 cores; replicate x.
  - The reference round-trips weight through fp8-e4m3, so every weight
    value is EXACTLY representable in TRN float8e4 (|w| <= ~5.5 << 240).
    Host converts the weight shard to fp8 and ships 8 MiB/core instead
    of 32 MiB -- a 4x cut of the dominant DMA stream.
  - x is shipped as fp16 (quantization rel err ~2e-4, budget 2e-2).
    The PE runs MIXED-dtype matmuls: fp16 stationary (x) x fp8 moving
    (weight), both upconverted to FP22 internally; fp8 streams at bf16
    rate so the ALU floor is 64 kt x 2 x 512 cycles ~ 27.5 us/core.
  - Host-side marshalling (layout only): weight shard transposed to
    [p, kt, o] and x packed to [p, kt, m] so every SBUF partition's DMA
    reads are contiguous DRAM runs.
  - scale/bias arrive as [1, O_shard] f32 rows, broadcast on-chip to the
    128 token partitions via an exact ones-outer-product on the (idle)
    PE.  Dequant scale is applied to the [128, O] output (64x fewer
    multiplies than dequantizing the weight), bias added, and the result
    is written back as fp16 (another ~1e-4 error, halves out traffic).
  - No collectives; the host concatenates the 8 output shards.

Memory floor per core: 8 MiB weight + 2 MiB x + 0.25 MiB out ~ 10.9 MiB
(~25 us at the 435 GB/s SBUF-AXI fabric ceiling, measured at line rate);
PE floor 128 matmuls x ~180-280 ns (clock-state dependent).  Measured
steady state ~28-35 us/invocation vs 87 us for the fp32 baseline.
"""

import numpy as np
import ml_dtypes

import concourse.bass as bass
import concourse.mybir as mybir
import concourse.tile as tile
from concourse.bass_utils import run_bass_kernel_spmd

B, S, IN, OUT = 4, 32, 8192, 8192
M = B * S                      # 128 tokens
NCORES = 8
OSH = OUT // NCORES            # 1024 out-features per core
KT = IN // 128                 # 64 k-tiles
F32 = mybir.dt.float32
F16 = mybir.dt.float16
F8 = mybir.dt.float8e4         # numpy side: ml_dtypes.float8_e4m3


def split_waits(nc, max_waits=1):
    """This walrus build encodes at most one sem-wait per instruction;
    move any excess onto NoOps injected just before (same engine queue,
    so ordering semantics are identical)."""
    n = 0
    for f in nc.m.functions:
        for bb in f.blocks:
            out = []
            for inst in bb.instructions:
                si = inst.sync_info
                waits = list(si.on_wait) if si and si.on_wait else []
                if len(waits) > max_waits:
                    extra, keep = waits[:-max_waits], waits[-max_waits:]
                    for i, w in enumerate(extra):
                        out.append(mybir.InstNoOp(
                            name=f"{inst.name}-ws{i}", engine=inst.engine,
                            ins=[], outs=[],
                            sync_info=mybir.SyncInfo(on_wait=[w], on_update=[])))
                        n += 1
                    si.on_wait = keep
                out.append(inst)
            bb.instructions = out
    return n


def _make_slabs(slab_kt):
    """Big slabs for DMA stream efficiency, tapered at the end so the final
    data->matmul->store dependency chain is short."""
    slabs = []
    k0 = 0
    while k0 < KT - slab_kt:
        slabs.append((k0, slab_kt))
        k0 += slab_kt
    while k0 < KT:
        n = max(1, min(4, KT - k0 - 2))
        slabs.append((k0, n))
        k0 += n
    return slabs


def _declare(nc):
    # host-packed layouts: each SBUF partition's DMA is one contiguous run
    xt_d = nc.dram_tensor("xt", [128, KT, M], F16, kind="ExternalInput")
    wt_d = nc.dram_tensor("wt", [128, KT, OSH], F8, kind="ExternalInput")
    sc_d = nc.dram_tensor("scale_r", [1, OSH], F32, kind="ExternalInput")
    bi_d = nc.dram_tensor("bias_r", [1, OSH], F32, kind="ExternalInput")
    out_d = nc.dram_tensor("out", [M, OSH], F16, kind="ExternalOutput")
    return xt_d, wt_d, sc_d, bi_d, out_d


def _emit_prologue(nc, cp, psb, x_eng, sc_d, bi_d):
    """Broadcast scale/bias rows to all 128 partitions on-chip: exact fp32
    outer product with a ones column on the (still idle) PE, instead of
    streaming 1 MiB of replicated data."""
    ones = cp.tile([1, M], F32)
    nc.vector.memset(ones[:], 1.0)
    sc = cp.tile([M, OSH], F32)
    bi = cp.tile([M, OSH], F32)
    for row_d, dst in ((sc_d, sc), (bi_d, bi)):
        row = cp.tile([1, OSH], F32, tag="crow")
        x_eng.dma_start(row[:], row_d[:])
        pb = psb.tile([M, OSH], F32, tag="pbcast")
        for og in range(2):
            nc.tensor.matmul(
                pb[:, og * 512:(og + 1) * 512],
                ones[:, :], row[:, og * 512:(og + 1) * 512],
                start=True, stop=True)
        nc.vector.tensor_copy(dst[:], pb[:])
    return sc, bi


def _emit_rep(nc, pools, engines, tensors, sc, bi, slabs, slab_kt, x_chunks):
    """One full shard computation: out[128, OSH] = (xT.T @ WT)*scale + bias.

    Queue discipline: ALL pure-prefetch traffic (x chunks + W slabs,
    interleaved in consumption order) goes on one HWDGE queue; the
    dependent out-DMAs go on the other.  HWDGE queues are FIFO per
    engine, so a compute-dependent DMA in the prefetch queue would stall
    the next rep's entire fill behind this rep's compute tail.
    """
    xp, wp, op, ps = pools
    stream_eng, out_eng = engines
    xt_d, wt_d, out_d = tensors

    xsb = xp.tile([128, KT, M], F16)
    x_per = KT // x_chunks

    def emit_x_chunk(i):
        stream_eng.dma_start(
            xsb[:, i * x_per:(i + 1) * x_per, :],
            xt_d[:, i * x_per:(i + 1) * x_per, :])

    # separate tags so consecutive reps alternate over 4 PSUM banks --
    # measured 177 ns/MM vs 283 ns/MM with a shared 2-bank rotation
    acc0 = ps.tile([M, 512], F32, tag="a0")
    acc1 = ps.tile([M, 512], F32, tag="a1")
    accs = (acc0, acc1)
    next_x = 0
    for t, (k0, n) in enumerate(slabs):
        # keep the x stream just ahead of the k-tiles that consume it
        while next_x < x_chunks and next_x * x_per <= k0 + n:
            emit_x_chunk(next_x)
            next_x += 1
        wsb = wp.tile([128, slab_kt, OSH], F8, tag="wsb")
        stream_eng.dma_start(wsb[:, :n, :], wt_d[:, k0:k0 + n, :])
        for s in range(n):
            k = k0 + s
            for og in range(2):
                nc.tensor.matmul(
                    accs[og][:, :],
                    xsb[:, k, :],
                    wsb[:, s, og * 512:(og + 1) * 512],
                    start=(k == 0), stop=(k == KT - 1))
    while next_x < x_chunks:
        emit_x_chunk(next_x)
        next_x += 1

    tmp = op.tile([M, OSH], F32, tag="tmp32")
    outsb = op.tile([M, OSH], F16, tag="o16")
    for og in range(2):
        osl = slice(og * 512, (og + 1) * 512)
        nc.vector.tensor_mul(tmp[:, osl], accs[og][:, :], sc[:, osl])
        nc.vector.tensor_add(outsb[:, osl], tmp[:, osl], bi[:, osl])
        # write each half back as soon as its scale/bias is done
        out_eng.dma_start(out_d[:, osl], outsb[:, osl])


def build(reps=1, slab_kt=4, stream_engine="scalar", out_engine="sync",
          x_chunks=8, loops=0, wp_bufs=8):
    """One column-parallel shard.

    reps > 1 unrolls the whole body (including all DMA) back-to-back for
    steady-state timing; the computation is identical each rep.
    loops > 0 wraps the reps in a hardware For_i loop executing the body
    `loops` times (for low-noise on-device timing; same data each pass).
    """
    nc = bass.Bass()
    xt_d, wt_d, sc_d, bi_d, out_d = _declare(nc)
    slabs = _make_slabs(slab_kt)

    with tile.TileContext(nc) as tc:
        with (
            tc.tile_pool(name="xp", bufs=2) as xp,
            tc.tile_pool(name="wp", bufs=wp_bufs) as wp,
            tc.tile_pool(name="cp", bufs=1) as cp,
            tc.tile_pool(name="op", bufs=2) as op,
            tc.tile_pool(name="psb", bufs=1, space="PSUM") as psb,
            tc.tile_pool(name="ps", bufs=2, space="PSUM") as ps,
        ):
            stream_eng = getattr(nc, stream_engine)
            out_eng = getattr(nc, out_engine)
            sc, bi = _emit_prologue(nc, cp, psb, out_eng, sc_d, bi_d)

            pools = (xp, wp, op, ps)
            engines = (stream_eng, out_eng)
            tensors = (xt_d, wt_d, out_d)

            def emit_reps():
                for _ in range(reps):
                    _emit_rep(nc, pools, engines, tensors, sc, bi,
                              slabs, slab_kt, x_chunks)

            if loops > 0:
                with tc.For_i(0, loops):
                    emit_reps()
            else:
                emit_reps()

    split_waits(nc)
    return nc


def shard_inputs(x, weight, weight_scale, bias):
    """Host-side marshalling into per-core input maps (layout + dtype only;
    the fp8 weight conversion is exact because the reference round-trips
    weight through fp8-e4m3)."""
    x = np.asarray(x, dtype=np.float32)
    weight = np.asarray(weight, dtype=np.float32)
    scale = np.asarray(weight_scale, dtype=np.float32).reshape(OUT)
    bias32 = np.asarray(bias).astype(np.float32)

    # pack x as [p, kt, m] (k = kt*128 + p) so each SBUF partition's x data
    # is one contiguous DRAM run
    xt = np.ascontiguousarray(
        np.transpose(x.reshape(M, KT, 128), (2, 1, 0))).astype(np.float16)
    in_maps = []
    for c in range(NCORES):
        sl = slice(c * OSH, (c + 1) * OSH)
        # wt[p, kt, o] = W_shard[o, kt*128 + p]  (k-major on partitions)
        wt = np.ascontiguousarray(
            weight[sl, :].T.reshape(KT, 128, OSH).transpose(1, 0, 2)
        ).astype(ml_dtypes.float8_e4m3)
        in_maps.append({
            "xt": xt, "wt": wt,
            "scale_r": np.ascontiguousarray(scale[sl][None, :]),
            "bias_r": np.ascontiguousarray(bias32[sl][None, :]),
        })
    return in_maps


def kernel(x, weight, weight_scale, bias):
    nc = build(reps=1)
    in_maps = shard_inputs(x, weight, weight_scale, bias)
    res = run_bass_kernel_spmd(nc, in_maps, core_ids=list(range(NCORES)))
    out = np.concatenate(
        [np.asarray(res.results[c]["out"]) for c in range(NCORES)], axis=1)
    return out.astype(np.float32).reshape(B, S, OUT)
